# revision 11
# baseline (speedup 1.0000x reference)
"""Two-layer GCN (GraphConv norm='both') on 8 Trainium2 NeuronCores.

v2 strategy (the baseline's critical path was GpSimd/Q7 SWDGE descriptor
generation for dma_gather: ~8.4ns/idx x 150k idx/core = ~1.17ms serial).

Key restructurings vs the baseline:
  1. W commutes out of the aggregation:  D^-1/2 A D^-1/2 (X W) =
     (D^-1/2 A D^-1/2 X) W.  Each layer aggregates RAW feature rows via
     one-hot selector matmuls (S carries norm_src[src]*norm_dst[dst] per
     edge), then applies W once per 128-node tile post-aggregation.
  2. Layer 1 therefore aggregates rows of X itself -- the host pre-expands
     x[src] into edge-chunk order (pure data layout / sharding prep) and the
     device just streams it.  Layer 1 needs NO device gather at all.
  3. Layer 2's gathers are gated as finely as possible: edge chunks whose
     sources all lie in the first RS=2560 rows of every core ("A-run")
     gather from a table filled by an early sub-AllGather of just those
     rows, so the Q7 descriptor stream starts ~100us in, right behind
     layer 1's first 16 tiles.  (A prepare_only/trigger_dma variant was
     measured slower: 86 triggers cost ~1.4us of engine time each.)
  4. Outputs are node-major; a single AllGather (of r) replaces the
     baseline's two.

Per-core Q7 work drops from 2x~583us to 1x~620us, and the span tracks the
L2 descriptor generation plus its start latency.
"""

import numpy as np

N_NODES = 50000
N_EDGES = 600000
D = 128
N_CORES = 8
NPC = N_NODES // N_CORES          # 6250 nodes per core
NT = (NPC + 127) // 128           # 49 dst tiles per core
RS = 20 * 128                     # A-half rows per core (2560)
RB = NPC - RS                     # B-half rows per core (3690)
W = 8                             # chunks per gather window (single-packet cap)
MT_BUFS = 20                      # gather window lookahead
BT = 4
TRIM = False

_CACHE = {}


def _schedule(sched):
    """Expand the shared (static, max/min-over-cores) schedule tuples into
    position-space layout. Position space: [A-run | A-pad | B-run | B-pad],
    where A-run holds the per-(tile,parity) complete chunks of A-half
    (src row < RS) edges and B-run the rest (incl. A/B straddle chunks)."""
    C1, Ctot, kA = (np.array(x) for x in sched)   # [NT], [NT,2], [NT,2]
    CB = Ctot - kA
    base1 = np.concatenate([[0], np.cumsum(C1)[:-1]])
    nchunk1 = int(C1.sum())

    a_base = np.zeros((NT, 2), dtype=np.int64)    # chunk offset in A-run
    b_base = np.zeros((NT, 2), dtype=np.int64)
    pos = 0
    for t in range(NT):
        for p in range(2):
            a_base[t, p] = pos
            pos += kA[t, p]
    nA = pos
    padA = (-nA) % W
    pos = nA + padA
    for t in range(NT):
        for p in range(2):
            b_base[t, p] = pos
            pos += CB[t, p]
    nB = pos - (nA + padA)
    padB = (-pos) % W
    nchunk2 = pos + padB
    nWA = (nA + padA) // W
    nW2 = nchunk2 // W
    return dict(C1=C1, Ctot=Ctot, kA=kA, CB=CB, base1=base1, nchunk1=nchunk1,
                a_base=a_base, b_base=b_base, nA=nA, padA=padA, nchunk2=nchunk2,
                nWA=nWA, nW2=nW2)


def _host_prep(x, src, dst, W1, b1, W2, b2):
    x = np.asarray(x, dtype=np.float32)
    src = np.asarray(src, dtype=np.int64)
    dst = np.asarray(dst, dtype=np.int64)
    W1 = np.asarray(W1, dtype=np.float32)
    W2 = np.asarray(W2, dtype=np.float32)
    b1 = np.asarray(b1, dtype=np.float32)
    b2 = np.asarray(b2, dtype=np.float32)

    deg_out = np.bincount(src, minlength=N_NODES).astype(np.float32)
    deg_in = np.bincount(dst, minlength=N_NODES).astype(np.float32)
    norm_src = np.where(deg_out > 0, 1.0 / np.sqrt(np.maximum(deg_out, 1.0)), 0.0)
    norm_dst = np.where(deg_in > 0, 1.0 / np.sqrt(np.maximum(deg_in, 1.0)), 0.0)
    sval = (norm_src[src] * norm_dst[dst]).astype(np.float32)
    x16 = x.astype(np.float16)

    # --- per-core edge grouping ---
    per_core = []
    cnt1 = np.zeros((N_CORES, NT), dtype=np.int64)
    cnt2 = np.zeros((N_CORES, NT * 2), dtype=np.int64)     # per (tile, parity)
    cntA = np.zeros((N_CORES, NT * 2), dtype=np.int64)     # A-half subset
    for k in range(N_CORES):
        m = (dst >= k * NPC) & (dst < (k + 1) * NPC)
        s_k = src[m]
        dl_k = dst[m] - k * NPC
        sv_k = sval[m]
        t_k = dl_k >> 7
        rs_k = s_k % NPC
        half = (rs_k >= RS).astype(np.int64)
        g2 = t_k * 2 + (s_k & 1)
        order = np.lexsort((s_k, half, g2))    # by (tile,par), then half, src
        s_k, dl_k, sv_k, g2, half = (a[order] for a in (s_k, dl_k, sv_k, g2, half))
        cnt1[k] = np.bincount(t_k, minlength=NT)
        cnt2[k] = np.bincount(g2, minlength=NT * 2)
        cntA[k] = np.bincount(g2[half == 0], minlength=NT * 2)
        per_core.append((s_k, dl_k, sv_k, g2))

    # --- shared static schedule ---
    C1 = np.maximum.reduce([(cnt1[k] + 127) // 128 for k in range(N_CORES)])
    C1 = np.maximum(C1, 1)
    Ctot = np.maximum.reduce([(cnt2[k] + 127) // 128 for k in range(N_CORES)])
    Ctot = np.maximum(Ctot, 1).reshape(NT, 2)
    kA = np.minimum.reduce([cntA[k] // 128 for k in range(N_CORES)]).reshape(NT, 2)
    kA = np.minimum(kA, Ctot)     # safety
    sched = (tuple(int(v) for v in C1),
             tuple(tuple(int(v) for v in row) for row in Ctot),
             tuple(tuple(int(v) for v in row) for row in kA))
    S = _schedule(sched)
    nchunk1, nchunk2 = S["nchunk1"], S["nchunk2"]
    meta = (nchunk1, nchunk2)

    a128 = (S["a_base"].reshape(-1) * 128)
    b128 = (S["b_base"].reshape(-1) * 128)
    kA128 = (kA.reshape(-1) * 128)
    base1_128 = S["base1"] * 128

    in_maps = []
    for k in range(N_CORES):
        s_k, dl_k, sv_k, g2 = per_core[k]
        t_k = dl_k >> 7

        # L2 positions: rank within (tile,par) group; first kA*128 slots go
        # to the A-run, the rest to the B-run.
        grp_counts = np.bincount(g2, minlength=NT * 2)
        grp_start = np.concatenate([[0], np.cumsum(grp_counts)[:-1]])
        rank = np.arange(len(g2)) - grp_start[g2]
        inA = rank < kA128[g2]
        pos2 = np.where(inA, a128[g2] + rank, b128[g2] + (rank - kA128[g2]))

        # gather index: row in H2A (== H2cat[:8*RS]) or H2cat B-region
        ks = s_k // NPC
        rs_k = s_k % NPC
        cat = np.where(rs_k < RS, ks * RS + rs_k,
                       N_CORES * RS + ks * RB + (rs_k - RS))
        idx16 = np.zeros(nchunk2 * 128, dtype=np.int16)
        filled = np.zeros(nchunk2 * 128, dtype=bool)
        idx16[pos2] = (cat >> 1).astype(np.int16)
        filled[pos2] = True
        # ucode drops trailing negative idxs per gather call: mark each
        # window's trailing pad run as -1 (S is zero there; mt keeps stale
        # finite data -- first MT_BUFS window tiles are memset on device).
        fw = filled.reshape(-1, W * 128)
        tail = np.cumsum(fw[:, ::-1], axis=1)[:, ::-1] == 0
        if TRIM:
            idx16[tail.reshape(-1)] = -1
        idx_wrapped = np.tile(idx16.reshape(-1, 16).T, (8, 1))

        S2 = np.zeros((128, nchunk2, 128), dtype=np.float16)
        S2[pos2 % 128, pos2 // 128, dl_k & 127] = sv_k.astype(np.float16)

        # L1 positions: rank within tile group (order within tile reuses the
        # (par, half, src) sort -- irrelevant for correctness).
        o1 = np.argsort(t_k, kind="stable")
        t1 = t_k[o1]
        grp_counts1 = np.bincount(t1, minlength=NT)
        grp_start1 = np.concatenate([[0], np.cumsum(grp_counts1)[:-1]])
        rank1 = np.arange(len(t1)) - grp_start1[t1]
        pos1 = base1_128[t1] + rank1

        yE = np.zeros((128, nchunk1, 128), dtype=np.float16)
        yE[pos1 % 128, pos1 // 128, :] = x16[s_k[o1]]
        S1 = np.zeros((128, nchunk1, 128), dtype=np.float16)
        S1[pos1 % 128, pos1 // 128, dl_k[o1] & 127] = sv_k[o1].astype(np.float16)

        in_maps.append(
            {
                "yE": np.ascontiguousarray(yE.reshape(128, nchunk1 * 128)),
                "S1": np.ascontiguousarray(S1.reshape(128, nchunk1 * 128)),
                "S2": np.ascontiguousarray(S2.reshape(128, nchunk2 * 128)),
                "idx_all": idx_wrapped,
                "W1f": W1.astype(np.float16),
                "W2f": W2.astype(np.float16),
                "B1bc": np.ascontiguousarray(
                    np.broadcast_to(b1, (128, 128)).astype(np.float32)),
                "B2bc": np.ascontiguousarray(
                    np.broadcast_to(b2, (128, 128)).astype(np.float32)),
            }
        )
    return in_maps, sched, meta


def _build_program(sched, meta):
    import concourse.bacc as bacc
    import concourse.mybir as mybir
    import concourse.tile as tile
    from concourse.library_config import mlp

    S = _schedule(sched)
    C1, Ctot, kA, CB = S["C1"], S["Ctot"], S["kA"], S["CB"]
    base1, nchunk1 = S["base1"], S["nchunk1"]
    a_base, b_base, nchunk2, nWA, nW2 = (
        S["a_base"], S["b_base"], S["nchunk2"], S["nWA"], S["nW2"])
    nW1 = (nchunk1 + W - 1) // W
    assert meta == (nchunk1, nchunk2)

    f16 = mybir.dt.float16
    f32 = mybir.dt.float32
    AF = mybir.ActivationFunctionType
    ALU = mybir.AluOpType

    nc = bacc.Bacc("TRN2", target_bir_lowering=False, debug=False,
                   num_devices=N_CORES, num_swdge_queues=4,
                   dynamic_dma_scratch_size=32768)

    yE_d = nc.dram_tensor("yE", [128, nchunk1 * 128], f16, kind="ExternalInput")
    S1_d = nc.dram_tensor("S1", [128, nchunk1 * 128], f16, kind="ExternalInput")
    S2_d = nc.dram_tensor("S2", [128, nchunk2 * 128], f16, kind="ExternalInput")
    idx_d = nc.dram_tensor("idx_all", [128, nchunk2 * 8], mybir.dt.int16,
                           kind="ExternalInput")
    W1_d = nc.dram_tensor("W1f", [128, 128], f16, kind="ExternalInput")
    W2_d = nc.dram_tensor("W2f", [128, 128], f16, kind="ExternalInput")
    B1_d = nc.dram_tensor("B1bc", [128, 128], f32, kind="ExternalInput")
    B2_d = nc.dram_tensor("B2bc", [128, 128], f32, kind="ExternalInput")

    rA_d = nc.dram_tensor("rA", [RS, D], f16, kind="Internal")
    rB_d = nc.dram_tensor("rB", [RB, D], f16, kind="Internal")
    H2A = nc.dram_tensor("H2A", [N_CORES * RS, D], f16, kind="Internal",
                         addr_space="Shared")
    H2C = nc.dram_tensor("H2C", [N_NODES, D], f16, kind="Internal",
                         addr_space="Shared")
    out_d = nc.dram_tensor("out", [NPC, D], f32, kind="ExternalOutput")

    with tile.TileContext(nc) as tc:
        with (
            tc.tile_pool(name="consts", bufs=1) as consts,
            tc.tile_pool(name="l1y", bufs=6) as l1y_pool,
            tc.tile_pool(name="l1s", bufs=6) as l1s_pool,
            tc.tile_pool(name="s2", bufs=6) as s2_pool,
            tc.tile_pool(name="mt", bufs=MT_BUFS) as mt_pool,
            tc.tile_pool(name="za", bufs=NT) as za_pool,
            tc.tile_pool(name="hb", bufs=10) as hb_pool,
            tc.tile_pool(name="psz", bufs=2, space="PSUM") as psz_pool,
            tc.tile_pool(name="psw", bufs=4, space="PSUM") as psw_pool,
        ):
            nc.gpsimd.load_library(mlp)

            W1f = consts.tile([128, 128], f16, tag="W1f")
            W2f = consts.tile([128, 128], f16, tag="W2f")
            B1bc = consts.tile([128, 128], f32, tag="B1bc")
            B2bc = consts.tile([128, 128], f32, tag="B2bc")
            idx_all = consts.tile([128, nchunk2 * 8], mybir.dt.int16, tag="idx")
            nc.sync.dma_start(W1f[:], W1_d.ap())
            nc.sync.dma_start(W2f[:], W2_d.ap())
            nc.sync.dma_start(B1bc[:], B1_d.ap())
            nc.sync.dma_start(B2bc[:], B2_d.ap())
            nc.sync.dma_start(idx_all[:], idx_d.ap())

            # ---------- writers (node-major row streams to DRAM) ----------
            def make_writer(dram, t_lo, t_hi, dt):
                nfull = min(t_hi, NPC // 128) - t_lo
                h3 = dram.ap()[0: nfull * 128, :].rearrange(
                    "(a p) d -> p a d", p=128)
                state = {}

                def write(t, produce):
                    tl_ = t - t_lo
                    if tl_ < nfull:
                        g = tl_ - tl_ % BT
                        if tl_ % BT == 0:
                            state["buf"] = hb_pool.tile(
                                [128, BT, 128], dt, tag=f"w{dt}", name="wstage")
                        produce(state["buf"][:, tl_ % BT, :])
                        if tl_ % BT == BT - 1 or tl_ == nfull - 1:
                            n = tl_ - g + 1
                            nc.sync.dma_start(h3[:, g: g + n, :],
                                              state["buf"][:, 0:n, :])
                    else:
                        rows = NPC - t * 128
                        tl = hb_pool.tile([128, 128], dt, tag=f"rag{dt}",
                                          name="wrag")
                        produce(tl[:])
                        nc.sync.dma_start(
                            dram.ap()[tl_ * 128: tl_ * 128 + rows, :],
                            tl[:rows, :])

                return write

            # ---------------- L1: z1 = S1.T @ yE, r = relu(z1@W1+b1) --------
            l1_tiles = {}

            def ensure1(w):
                if w in l1_tiles:
                    return l1_tiles[w]
                cb = w * W
                cw = min(W, nchunk1 - cb)
                yt = l1y_pool.tile([128, cw * 128], f16, tag="yt")
                nc.sync.dma_start(yt[:], yE_d.ap()[:, cb * 128:(cb + cw) * 128])
                st = l1s_pool.tile([128, cw * 128], f16, tag="s1t")
                nc.scalar.dma_start(st[:], S1_d.ap()[:, cb * 128:(cb + cw) * 128])
                l1_tiles[w] = (yt, st)
                return yt, st

            wr_rA = make_writer(rA_d, 0, RS // 128, f16)
            wr_rB = make_writer(rB_d, RS // 128, NT, f16)

            def sub_ag(src_dram, dst_ap):
                nc.gpsimd.collective_compute(
                    "AllGather", ALU.bypass,
                    replica_groups=[list(range(N_CORES))],
                    ins=[src_dram.ap()], outs=[dst_ap],
                )

            for t in range(NT):
                zp = psz_pool.tile([128, 128], f32, tag="zp", name="z1")
                c0, c1 = int(base1[t]), int(base1[t] + C1[t])
                for c in range(c0, c1):
                    yt, st = ensure1(c // W)
                    o = c % W
                    nc.tensor.matmul(zp[:], yt[:, o * 128:(o + 1) * 128],
                                     st[:, o * 128:(o + 1) * 128],
                                     start=(c == c0), stop=(c == c1 - 1))
                z1sb = hb_pool.tile([128, 128], f16, tag="zsb", name="z1sb")
                nc.scalar.activation(z1sb[:], zp[:], AF.Copy)
                pw = psw_pool.tile([128, 128], f32, tag="pw", name="pw1")
                nc.tensor.matmul(pw[:], z1sb[:], W1f[:])

                def produce_r(dst, pw=pw):
                    rt = hb_pool.tile([128, 128], f16, tag="rt", name="rt")
                    nc.vector.tensor_tensor(rt[:], pw[:], B1bc[:], op=ALU.add)
                    nc.vector.tensor_scalar_max(dst, rt[:], 0.0)

                (wr_rA if t < RS // 128 else wr_rB)(t, produce_r)
                if t == RS // 128 - 1:
                    sub_ag(rA_d, H2A.ap())
                    sub_ag(rA_d, H2C.ap()[0: N_CORES * RS, :])
            sub_ag(rB_d, H2C.ap()[N_CORES * RS: N_NODES, :])

            # ---------------- L2: gathers + two-pass agg ----------
            HA_pairs = H2A.ap().rearrange("(a b) d -> a (b d)", b=2)
            HC_pairs = H2C.ap().rearrange("(a b) d -> a (b d)", b=2)
            mt_tiles = {}

            def ensure2(w):
                if w in mt_tiles:
                    return mt_tiles[w]
                cb = w * W
                q = w % 4
                mt = mt_pool.tile([128, W, 256], f16, tag="mt")
                src = HA_pairs if w < nWA else HC_pairs
                nc.gpsimd.dma_gather(
                    mt[:], src, idx_all[:, cb * 8:(cb + W) * 8],
                    W * 128, W * 128, 256, queue_num=q)
                st = s2_pool.tile([128, W * 128], f16, tag="s2t")
                nc.scalar.dma_start(st[:], S2_d.ap()[:, cb * 128:(cb + W) * 128])
                mt_tiles[w] = (mt, st)
                return mt, st

            def agg_run(t, crange_list):
                """Accumulate the chunks of tile t (list of (chunk, par))."""
                pa = psz_pool.tile([128, 128], f32, tag="zp", name="z2")
                n = len(crange_list)
                for i, (c, par) in enumerate(crange_list):
                    mt, st = ensure2(c // W)
                    o = c % W
                    nc.tensor.matmul(
                        pa[:], mt[:, o, par * 128: par * 128 + 128],
                        st[:, o * 128:(o + 1) * 128],
                        start=(i == 0), stop=(i == n - 1))
                return pa

            def chunks_of(t, base, cnt):
                out = []
                for p in range(2):
                    out += [(int(base[t, p]) + j, p) for j in range(int(cnt[t, p]))]
                return out

            # pass A
            zA = {}
            for t in range(NT):
                cl = chunks_of(t, a_base, kA)
                zA_t = za_pool.tile([128, 128], f16, tag="zA", name="zA")
                if cl:
                    pa = agg_run(t, cl)
                    nc.scalar.activation(zA_t[:], pa[:], AF.Copy)
                else:
                    nc.vector.memset(zA_t[:], 0.0)
                zA[t] = zA_t

            # pass B
            wr_out = make_writer(out_d, 0, NT, f32)
            for t in range(NT):
                cl = chunks_of(t, b_base, CB)
                if cl:
                    pb = agg_run(t, cl)
                    z2sb = hb_pool.tile([128, 128], f16, tag="zsb", name="z2sb")
                    nc.vector.tensor_tensor(z2sb[:], pb[:], zA[t][:], op=ALU.add)
                else:
                    z2sb = zA[t]
                pw2 = psw_pool.tile([128, 128], f32, tag="pw", name="pw2")
                nc.tensor.matmul(pw2[:], z2sb[:], W2f[:])
                wr_out(t, lambda dst, pw2=pw2: nc.vector.tensor_tensor(
                    dst, pw2[:], B2bc[:], op=ALU.add))

    nc.compile()
    return nc


def kernel(x, src, dst, W1, b1, W2, b2):
    from concourse.bass_utils import run_bass_kernel_spmd

    in_maps, sched, meta = _host_prep(x, src, dst, W1, b1, W2, b2)
    key = (sched, meta)
    if key not in _CACHE:
        _CACHE[key] = _build_program(sched, meta)
    nc = _CACHE[key]
    res = run_bass_kernel_spmd(nc, in_maps, core_ids=list(range(N_CORES)))
    out = np.empty((N_NODES, D), dtype=np.float32)
    for k in range(N_CORES):
        out[k * NPC: (k + 1) * NPC] = res.results[k]["out"]
    return out
